# revision 8
# baseline (speedup 1.0000x reference)
"""EngramEmbeddings Trainium2 kernel.

Expert-sharded across 8 NeuronCores: core c owns head c of both the n=2 and
n=3 hash tables (concatenated into one DRAM table per core) and computes the
hashed-ngram embedding lookup for all B*S = 32768 tokens for its two slots.

Device-side work per core:
  1. int64 hash (id*seed per ngram term, XOR, mod table_size) computed exactly
     with 16-bit limb arithmetic on the vector engine (DVE arithmetic is fp32
     internally, so every arithmetic intermediate is kept < 2^24; bit surgery
     uses exact int32 bitwise/shift ops).
  2. indirect-DMA gather of 80-float rows from the table (4096 rows per
     instruction) into SBUF, assembling [128, TT*160] tiles.
  3. contiguous store of the per-core output [32768, 160].

Host does only sharding-style prep: dtype casts, per-batch-row shift/padding
of ids, splitting the (runtime input) seeds into 16-bit halves, and the
table concat; plus the final np stitch of per-core outputs.
"""

import numpy as np

try:
    import concourse  # noqa: F401
except ImportError:  # pragma: no cover
    import sys

    for _p in ("/opt/trn_rl_repo", "/root/.axon_site/_ro/trn_rl_repo"):
        if _p not in sys.path:
            sys.path.insert(0, _p)

import concourse.bass as bass
import concourse.tile as tile
from concourse import bacc, mybir
from concourse.bass_utils import run_bass_kernel_spmd

N2_SIZES = [6619, 6637, 6653, 6659, 6661, 6673, 6679, 6689]
N3_SIZES = [65521, 65537, 65539, 65543, 65551, 65557, 65563, 65579]
B, S = 8, 4096
P = 128
NTOK = B * S            # 32768
TPB = NTOK // P         # 256 tokens per partition
NPAIR = 5               # (prv,s0)(cur,s1) for n2 + (pv2,s0)(prv,s1)(cur,s2) for n3
WID = NPAIR * TPB       # 1280
NSLOT = 2               # [n2, n3] per core
SW = NSLOT * TPB        # 512
SLOT = 80
D = NSLOT * SLOT        # 160
TT = 16                 # tokens per partition per gather iteration
NIT = TPB // TT
VMAX = max(a + b for a, b in zip(N2_SIZES, N3_SIZES))  # 72268

_NC = None
TRACE = False
LAST_RESULT = None


def _build_nc():
    dt = mybir.dt
    A = mybir.AluOpType
    AND, XOR = A.bitwise_and, A.bitwise_xor
    LSR, LSL = A.logical_shift_right, A.logical_shift_left
    ADD, MULT, SUB, GE = A.add, A.mult, A.subtract, A.is_ge

    nc = bacc.Bacc("TRN2", target_bir_lowering=False, debug=False)
    tbl = nc.dram_tensor("tbl", [VMAX, SLOT], dt.float32, kind="ExternalInput")
    ids3 = nc.dram_tensor("ids3", [3, NTOK], dt.int32, kind="ExternalInput")
    s0d = nc.dram_tensor("s0w", [P, WID], dt.int32, kind="ExternalInput")
    s1d = nc.dram_tensor("s1w", [P, WID], dt.int32, kind="ExternalInput")
    cstd = nc.dram_tensor("cst", [P, 6 * SW], dt.int32, kind="ExternalInput")
    invd = nc.dram_tensor("invc", [P, SW], dt.float32, kind="ExternalInput")
    outd = nc.dram_tensor("out", [NTOK, D], dt.float32, kind="ExternalOutput")
    dbgd = nc.dram_tensor("dbg", [P, 10 * SW], dt.int32, kind="ExternalOutput")

    with tile.TileContext(nc) as tc:
        with (
            tc.tile_pool(name="c", bufs=1) as cp,
            tc.tile_pool(name="w", bufs=1) as wp,
            tc.tile_pool(name="s", bufs=1) as sp,
            tc.tile_pool(name="g", bufs=3) as gp,
        ):
            i32 = dt.int32

            def wt(tag=None):
                # unnamed temps rotate through 6 shared slots; named tiles
                # (single-instance) get their own slot
                if tag is None:
                    return wp.tile([P, WID], i32, tag="wtmp", bufs=6,
                                   name=f"w{nc.next_id()}")
                return wp.tile([P, WID], i32, tag=tag, bufs=1,
                               name=f"w{nc.next_id()}")

            def st(tag=None):
                if tag is None:
                    return sp.tile([P, SW], i32, tag="stmp", bufs=8,
                                   name=f"s{nc.next_id()}")
                return sp.tile([P, SW], i32, tag=tag, bufs=1,
                               name=f"s{nc.next_id()}")

            # --- constants / inputs to SBUF ---
            s0w = cp.tile([P, WID], i32, tag="s0w")
            nc.sync.dma_start(s0w[:], s0d.ap())
            s1w = cp.tile([P, WID], i32, tag="s1w")
            nc.sync.dma_start(s1w[:], s1d.ap())
            cst = cp.tile([P, 6 * SW], i32, tag="cst")
            nc.sync.dma_start(cst[:], cstd.ap())
            Mt = cst[:, 0 * SW : 1 * SW]
            R16 = cst[:, 1 * SW : 2 * SW]
            R24 = cst[:, 2 * SW : 3 * SW]
            R32 = cst[:, 3 * SW : 4 * SW]
            R40 = cst[:, 4 * SW : 5 * SW]
            BASE = cst[:, 5 * SW : 6 * SW]

            INV = cp.tile([P, SW], dt.float32, tag="inv")
            nc.sync.dma_start(INV[:], invd.ap())

            def mod_m(x):
                """x mod m for 0 <= x < 2^24 (m = Mt per column), exact.

                q = round_nearest(x*inv_lo - 0.5) is in {floor(x/m)-1,
                floor(x/m)} because inv_lo is biased ~1e-6 low, so
                q*m <= x < 2^24 stays fp32-exact and r = x - q*m is in
                [0, 2m); one conditional subtract of m finishes it."""
                y = sp.tile([P, SW], dt.float32, tag="modf", bufs=2,
                            name=f"y{nc.next_id()}")
                nc.vector.tensor_tensor(y[:], x[:], INV[:], MULT)
                y2 = sp.tile([P, SW], dt.float32, tag="modf2", bufs=2,
                             name=f"y2{nc.next_id()}")
                nc.vector.tensor_scalar(y2[:], y[:], 0.5, None, SUB)
                q = st()
                nc.vector.tensor_copy(q[:], y2[:])
                qm = st()
                nc.vector.tensor_tensor(qm[:], q[:], Mt, MULT)
                r = st()
                nc.vector.tensor_tensor(r[:], x[:], qm[:], SUB)
                ge = st()
                nc.vector.tensor_tensor(ge[:], r[:], Mt, GE)
                gm = st()
                nc.vector.tensor_tensor(gm[:], ge[:], Mt, MULT)
                r2 = st()
                nc.vector.tensor_tensor(r2[:], r[:], gm[:], SUB)
                return r2

            idsv = ids3.ap().rearrange("r (p t) -> r p t", p=P)
            idst = []
            for r in range(3):
                t_ = cp.tile([P, TPB], i32, tag=f"ids{r}")
                nc.sync.dma_start(t_[:], idsv[r])
                idst.append(t_)
            prv, pv2, cur = idst

            # --- X: token ids replicated per (term, seed) pair block ---
            X = wt("X")
            for j, s_ in enumerate([prv, cur, pv2, prv, cur]):
                nc.any.tensor_copy(X[:, j * TPB : (j + 1) * TPB], s_[:])

            # --- products via 8bit x 16bit partial products (all < 2^24) ---
            a0 = wt("a0")
            nc.vector.tensor_scalar(a0[:], X[:], 0xFF, None, AND)
            a1 = wt("a1")
            nc.vector.tensor_scalar(a1[:], X[:], 8, None, LSR)
            t00 = wt("t00")
            nc.vector.tensor_tensor(t00[:], a0[:], s0w[:], MULT)
            t10 = wt("t10")
            nc.vector.tensor_tensor(t10[:], a1[:], s0w[:], MULT)
            t01 = wt("t01")
            nc.vector.tensor_tensor(t01[:], a0[:], s1w[:], MULT)
            t11 = wt("t11")
            nc.vector.tensor_tensor(t11[:], a1[:], s1w[:], MULT)

            # limbs of p = t00 + t10*2^8 + t01*2^16 + t11*2^24  (p < 2^48)
            # walrus forbids mixing bitwise and arith ALU ops in one fused
            # instruction, so bit surgery and adds are separate ops
            Ap = wt()
            nc.vector.tensor_scalar(Ap[:], t10[:], 0xFF, 8, AND, LSL)
            v0a = wt()
            nc.vector.tensor_scalar(v0a[:], t00[:], 0xFFFF, None, AND)
            v0 = wt()
            nc.vector.tensor_tensor(v0[:], v0a[:], Ap[:], ADD)
            L0 = wt("L0")
            nc.vector.tensor_scalar(L0[:], v0[:], 0xFFFF, None, AND)
            c0 = wt()
            nc.vector.tensor_scalar(c0[:], v0[:], 16, None, LSR)
            u1a = wt()
            nc.vector.tensor_scalar(u1a[:], t10[:], 8, None, LSR)
            u1 = wt()
            nc.vector.tensor_tensor(u1[:], u1a[:], c0[:], ADD)
            u2a = wt()
            nc.vector.tensor_scalar(u2a[:], t01[:], 0xFFFF, None, AND)
            u2 = wt()
            nc.vector.tensor_tensor(u2[:], u2a[:], u1[:], ADD)
            u3a = wt()
            nc.vector.tensor_scalar(u3a[:], t00[:], 16, None, LSR)
            u3 = wt()
            nc.vector.tensor_tensor(u3[:], u3a[:], u2[:], ADD)
            Ff = wt()
            nc.vector.tensor_scalar(Ff[:], t11[:], 0xFF, 8, AND, LSL)
            v1 = wt()
            nc.vector.tensor_tensor(v1[:], u3[:], Ff[:], ADD)
            L1 = wt("L1")
            nc.vector.tensor_scalar(L1[:], v1[:], 0xFFFF, None, AND)
            c1 = wt()
            nc.vector.tensor_scalar(c1[:], v1[:], 16, None, LSR)
            v2a = wt()
            nc.vector.tensor_scalar(v2a[:], t01[:], 16, None, LSR)
            v2 = wt()
            nc.vector.tensor_tensor(v2[:], v2a[:], c1[:], ADD)
            L2a = wt()
            nc.vector.tensor_scalar(L2a[:], t11[:], 8, None, LSR)
            L2 = wt("L2")
            nc.vector.tensor_tensor(L2[:], L2a[:], v2[:], ADD)

            # --- XOR terms into per-slot limbs H* [P, SW] = [n2 | n3] ---
            b = [slice(j * TPB, (j + 1) * TPB) for j in range(NPAIR)]
            H0, H1, H2 = st("H0"), st("H1"), st("H2")
            for Ht, Lt in ((H0, L0), (H1, L1), (H2, L2)):
                nc.vector.tensor_tensor(Ht[:, :TPB], Lt[:, b[0]], Lt[:, b[1]], XOR)
                tmp = sp.tile([P, TPB], i32, tag="xtmp", bufs=2, name=f"xt{nc.next_id()}")
                nc.vector.tensor_tensor(tmp[:], Lt[:, b[2]], Lt[:, b[3]], XOR)
                nc.vector.tensor_tensor(Ht[:, TPB:], tmp[:], Lt[:, b[4]], XOR)

            # --- mod: h = H0 + H1*2^16 + H2*2^32 (mod m), via 8-bit pieces ---
            H1a = st()
            nc.vector.tensor_scalar(H1a[:], H1[:], 0xFF, None, AND)
            H1b = st()
            nc.vector.tensor_scalar(H1b[:], H1[:], 8, None, LSR)
            H2a = st()
            nc.vector.tensor_scalar(H2a[:], H2[:], 0xFF, None, AND)
            H2b = st()
            nc.vector.tensor_scalar(H2b[:], H2[:], 8, None, LSR)
            p1, p2, p3, p4 = st(), st(), st(), st()
            nc.vector.tensor_tensor(p1[:], H1a[:], R16, MULT)
            nc.vector.tensor_tensor(p2[:], H1b[:], R24, MULT)
            nc.vector.tensor_tensor(p3[:], H2a[:], R32, MULT)
            nc.vector.tensor_tensor(p4[:], H2b[:], R40, MULT)
            m1 = mod_m(p1)
            m2 = mod_m(p2)
            m3 = mod_m(p3)
            m4 = mod_m(p4)
            x1 = st()
            nc.vector.tensor_tensor(x1[:], H0[:], m1[:], ADD)
            x2 = st()
            nc.vector.tensor_tensor(x2[:], m2[:], m3[:], ADD)
            x3 = st()
            nc.vector.tensor_tensor(x3[:], x1[:], x2[:], ADD)
            x4 = st()
            nc.vector.tensor_tensor(x4[:], x3[:], m4[:], ADD)
            idx0 = mod_m(x4)
            idx = st("idx")
            nc.vector.tensor_tensor(idx[:], idx0[:], BASE, ADD)

            # debug dump: final idx + H limbs + mod intermediates
            nc.sync.dma_start(dbgd.ap()[:, 0:SW], idx[:])
            nc.sync.dma_start(dbgd.ap()[:, SW : 2 * SW], H0[:])
            nc.sync.dma_start(dbgd.ap()[:, 2 * SW : 3 * SW], H1[:])
            nc.sync.dma_start(dbgd.ap()[:, 3 * SW : 4 * SW], H2[:])
            nc.sync.dma_start(dbgd.ap()[:, 4 * SW : 5 * SW], m1[:])
            nc.sync.dma_start(dbgd.ap()[:, 5 * SW : 6 * SW], m2[:])
            nc.sync.dma_start(dbgd.ap()[:, 6 * SW : 7 * SW], m3[:])
            nc.sync.dma_start(dbgd.ap()[:, 7 * SW : 8 * SW], m4[:])
            nc.sync.dma_start(dbgd.ap()[:, 8 * SW : 9 * SW], x4[:])
            nc.sync.dma_start(dbgd.ap()[:, 9 * SW : 10 * SW], idx0[:])

            # --- gather + store ---
            # HW indirect DMA consumes exactly one runtime index per
            # partition per instruction, so each instruction gathers 128
            # rows ([P, 80]); accumulate TT tokens x 2 slots per store.
            outv = outd.ap().rearrange("(p t) d -> p t d", p=P)
            for it in range(NIT):
                g = gp.tile([P, TT * D], dt.float32, tag="g", name=f"g{it}")
                for tt in range(TT):
                    col = it * TT + tt
                    for s in range(NSLOT):
                        nc.gpsimd.indirect_dma_start(
                            out=g[:, tt * D + s * SLOT : tt * D + (s + 1) * SLOT],
                            out_offset=None,
                            in_=tbl.ap(),
                            in_offset=bass.IndirectOffsetOnAxis(
                                ap=idx[:, s * TPB + col : s * TPB + col + 1],
                                axis=0,
                            ),
                        )
                nc.sync.dma_start(
                    outv[:, it * TT : (it + 1) * TT, :],
                    g[:].rearrange("p (t d) -> p t d", d=D),
                )

    nc.compile()
    return nc


def _get_nc():
    global _NC
    if _NC is None:
        _NC = _build_nc()
    return _NC


def _make_in_maps(inputs):
    ids = np.asarray(inputs["canonical_ids"]).astype(np.int32)  # [B, S]
    hs = np.asarray(inputs["hash_seeds"]).astype(np.int64)      # [3, 8]
    cur = ids.reshape(-1)
    prv = np.pad(ids, ((0, 0), (1, 0)))[:, :S].reshape(-1)
    pv2 = np.pad(ids, ((0, 0), (2, 0)))[:, :S].reshape(-1)
    ids3 = np.ascontiguousarray(np.stack([prv, pv2, cur]).astype(np.int32))

    maps = []
    for c in range(8):
        s0, s1, s2 = int(hs[0, c]), int(hs[1, c]), int(hs[2, c])
        pair_seeds = [s0, s1, s0, s1, s2]
        s0row = np.concatenate(
            [np.full(TPB, sd & 0xFFFF, np.int32) for sd in pair_seeds]
        )
        s1row = np.concatenate([np.full(TPB, sd >> 16, np.int32) for sd in pair_seeds])
        m2, m3 = N2_SIZES[c], N3_SIZES[c]

        def row2(f):
            return np.concatenate(
                [np.full(TPB, f(m2), np.int32), np.full(TPB, f(m3), np.int32)]
            )

        cstrow = np.concatenate(
            [
                row2(lambda m: m),
                row2(lambda m: 2**16 % m),
                row2(lambda m: 2**24 % m),
                row2(lambda m: 2**32 % m),
                row2(lambda m: 2**40 % m),
                np.concatenate(
                    [np.zeros(TPB, np.int32), np.full(TPB, m2, np.int32)]
                ),
            ]
        )
        invrow = np.concatenate(
            [
                np.full(TPB, np.float64(1.0 / m2) * (1 - 1e-6), np.float32),
                np.full(TPB, np.float64(1.0 / m3) * (1 - 1e-6), np.float32),
            ]
        )
        tblc = np.zeros((VMAX, SLOT), np.float32)
        tblc[:m2] = np.asarray(inputs[f"w_n2_h{c}"], dtype=np.float32)
        tblc[m2 : m2 + m3] = np.asarray(inputs[f"w_n3_h{c}"], dtype=np.float32)
        maps.append(
            {
                "tbl": tblc,
                "ids3": ids3,
                "s0w": np.ascontiguousarray(np.broadcast_to(s0row, (P, WID))),
                "s1w": np.ascontiguousarray(np.broadcast_to(s1row, (P, WID))),
                "cst": np.ascontiguousarray(np.broadcast_to(cstrow, (P, 6 * SW))),
                "invc": np.ascontiguousarray(np.broadcast_to(invrow, (P, SW))),
            }
        )
    return maps


def kernel(**inputs):
    global LAST_RESULT
    nc = _get_nc()
    in_maps = _make_in_maps(inputs)
    res = run_bass_kernel_spmd(
        nc, in_maps, core_ids=list(range(8)), trace=TRACE
    )
    LAST_RESULT = res
    out = np.empty((B, S, 16 * SLOT), np.float32)
    for c in range(8):
        oc = res.results[c]["out"].reshape(P, TPB, NSLOT, SLOT)
        # token t = p*TPB + tt maps to flat b*S + s
        oc = oc.reshape(NTOK, NSLOT, SLOT).reshape(B, S, NSLOT, SLOT)
        out[:, :, c * SLOT : (c + 1) * SLOT] = oc[:, :, 0, :]
        out[:, :, (8 + c) * SLOT : (9 + c) * SLOT] = oc[:, :, 1, :]
    return out


# revision 10
# speedup vs baseline: 1.1621x; 1.1621x over previous
"""EngramEmbeddings Trainium2 kernel.

Expert-sharded across 8 NeuronCores: core c owns head c of the n=2 and n=3
hash tables and computes the hashed-ngram embedding lookup for all
B*S = 32768 tokens for its two slots.

Device-side work per core:
  1. int64 hash (id*seed per ngram term, XOR, mod table_size) computed
     exactly with 16-bit limb arithmetic on the vector engine (DVE
     arithmetic is fp32 internally, so every arithmetic intermediate stays
     < 2^24; bit surgery uses exact int32 bitwise/shift ops; mod is
     reciprocal-multiply + floor + one conditional subtract, exact).
  2. n2 slot (table < 32768 rows): dma_gather (fast Q7 ucode, int16
     indices, 512B-padded rows) — 8 instructions of 4096 rows each.
     Tokens are host-permuted into the ucode's wrapped stream order so
     gathered rows land p-major for contiguous stores.
  3. n3 slot (table > 32768 rows, exceeds dma_gather's int16 reach):
     indirect DMA, one 128-row instruction per token column.

Host does sharding-style prep only: dtype casts, per-batch-row
shift/padding of ids, token-order permutations, splitting the runtime
seeds into 16-bit halves, table concat/pad, and final output stitching.
"""

import numpy as np

try:
    import concourse  # noqa: F401
except ImportError:  # pragma: no cover
    import sys

    for _p in ("/opt/trn_rl_repo", "/root/.axon_site/_ro/trn_rl_repo"):
        if _p not in sys.path:
            sys.path.insert(0, _p)

import concourse.bass as bass
import concourse.tile as tile
from concourse import bacc, mybir
from concourse.bass_utils import run_bass_kernel_spmd

N2_SIZES = [6619, 6637, 6653, 6659, 6661, 6673, 6679, 6689]
N3_SIZES = [65521, 65537, 65539, 65543, 65551, 65557, 65563, 65579]
B, S = 8, 4096
P = 128
NTOK = B * S              # 32768
TPB = NTOK // P           # 256 tokens per partition (p-major: token = p*256+t)
SLOT = 80
V2 = max(N2_SIZES)        # 6689
V3 = max(N3_SIZES)        # 65579
E2 = 128                  # n2 table row padded to 128 f32 = 512B for dma_gather
NCH = 8                   # n2 dma_gather chunks (4096 tokens each)
CW = TPB // NCH           # 32 token-columns per chunk
N3C = 4                   # n3 hash computed in 4 slabs of 64 columns
TT3 = 32                  # n3 gather columns per store tile

_NC = None
TRACE = False
LAST_RESULT = None

# token permutation for the n2 dma_gather stream: chunk a's stream position
# j = c*16 + q (ucode wrapped order: lane q = j%16, col c = j//16) gathers
# the token landing at dest (partition u = j%128, block b = j//128), which
# we choose to be p-major token u*256 + 32a + b.  Hash lane (pi = 16a+q, c)
# therefore holds token TAU2[pi, c].
_c = np.arange(TPB)[None, :]
_pi = np.arange(P)[:, None]
TAU2 = ((16 * (_c % 8) + _pi % 16) * 256 + 32 * (_pi // 16) + _c // 8).astype(
    np.int64
)


def _build_nc():
    dt = mybir.dt
    A = mybir.AluOpType
    AND, XOR = A.bitwise_and, A.bitwise_xor
    LSR, LSL = A.logical_shift_right, A.logical_shift_left
    ADD, MULT, SUB, GE = A.add, A.mult, A.subtract, A.is_ge
    i32 = dt.int32
    f32 = dt.float32

    nc = bacc.Bacc("TRN2", target_bir_lowering=False, debug=False)
    tbl3 = nc.dram_tensor("tbl3", [V3, SLOT], f32, kind="ExternalInput")
    tbl2 = nc.dram_tensor("tbl2", [V2, E2], f32, kind="ExternalInput")
    ids3d = nc.dram_tensor("ids3", [3, NTOK], i32, kind="ExternalInput")
    ids2d = nc.dram_tensor("ids2", [2, NTOK], i32, kind="ExternalInput")
    s03d = nc.dram_tensor("s0w3", [P, 3 * TPB], i32, kind="ExternalInput")
    s13d = nc.dram_tensor("s1w3", [P, 3 * TPB], i32, kind="ExternalInput")
    s02d = nc.dram_tensor("s0w2", [P, 2 * TPB], i32, kind="ExternalInput")
    s12d = nc.dram_tensor("s1w2", [P, 2 * TPB], i32, kind="ExternalInput")
    # per-slot mod constants: M, R16, R24, R32, R40 (int32) + inv (f32)
    cst3d = nc.dram_tensor("cst3", [P, 5 * TPB], i32, kind="ExternalInput")
    cst2d = nc.dram_tensor("cst2", [P, 5 * TPB], i32, kind="ExternalInput")
    inv3d = nc.dram_tensor("inv3", [P, TPB], f32, kind="ExternalInput")
    inv2d = nc.dram_tensor("inv2", [P, TPB], f32, kind="ExternalInput")
    out3d = nc.dram_tensor("out3", [NTOK, SLOT], f32, kind="ExternalOutput")
    out2d = nc.dram_tensor("out2", [NTOK, E2], f32, kind="ExternalOutput")

    with tile.TileContext(nc) as tc:
        with (
            tc.tile_pool(name="c", bufs=1) as cp,
            tc.tile_pool(name="w", bufs=1) as wp,
            tc.tile_pool(name="g", bufs=1) as gp,
        ):

            def ld(dram, shape, dtype, tag):
                t = cp.tile(shape, dtype, tag=tag, name=tag)
                nc.sync.dma_start(t[:], dram.ap())
                return t

            s0w3 = ld(s03d, [P, 3 * TPB], i32, "s0w3")
            s1w3 = ld(s13d, [P, 3 * TPB], i32, "s1w3")
            s0w2 = ld(s02d, [P, 2 * TPB], i32, "s0w2")
            s1w2 = ld(s12d, [P, 2 * TPB], i32, "s1w2")
            cst3 = ld(cst3d, [P, 5 * TPB], i32, "cst3")
            cst2 = ld(cst2d, [P, 5 * TPB], i32, "cst2")
            inv3 = ld(inv3d, [P, TPB], f32, "inv3")
            inv2 = ld(inv2d, [P, TPB], f32, "inv2")

            ids3v = ids3d.ap().rearrange("r (p t) -> r p t", p=P)
            ids2v = ids2d.ap().rearrange("r (p t) -> r p t", p=P)
            id3 = []
            for r in range(3):
                t_ = cp.tile([P, TPB], i32, tag=f"id3_{r}", name=f"id3_{r}")
                nc.sync.dma_start(t_[:], ids3v[r])
                id3.append(t_)
            pv2_3, prv_3, cur_3 = id3  # rows: 0=pv2, 1=prv, 2=cur
            id2 = []
            for r in range(2):
                t_ = cp.tile([P, TPB], i32, tag=f"id2_{r}", name=f"id2_{r}")
                nc.sync.dma_start(t_[:], ids2v[r])
                id2.append(t_)
            prv_2, cur_2 = id2  # rows: 0=prv, 1=cur

            def hash_idx(srcs, s0w, s1w, cst, inv, C, col0, tagp):
                """Hashed table index for one slot over C token columns.

                srcs: list of id tiles (one per ngram term, cols col0..+C
                used).  All wide ops on [P, npair*C]; returns int32 idx tile
                [P, C] with values in [0, m).
                """
                npair = len(srcs)
                W = npair * C

                def wt():
                    return wp.tile([P, W], i32, tag=f"w{tagp}", bufs=8,
                                   name=f"w{tagp}_{nc.next_id()}")

                def st(dtype=i32):
                    return wp.tile([P, C], dtype, tag=f"s{tagp}{dtype}",
                                   bufs=10, name=f"s{tagp}_{nc.next_id()}")

                sl = [slice(j * C, (j + 1) * C) for j in range(npair)]
                cs = slice(col0, col0 + C)
                Mt = cst[:, 0 * TPB + col0 : 0 * TPB + col0 + C]
                R16 = cst[:, 1 * TPB + col0 : 1 * TPB + col0 + C]
                R24 = cst[:, 2 * TPB + col0 : 2 * TPB + col0 + C]
                R32 = cst[:, 3 * TPB + col0 : 3 * TPB + col0 + C]
                R40 = cst[:, 4 * TPB + col0 : 4 * TPB + col0 + C]
                INV = inv[:, cs]
                s0v = [s0w[:, j * TPB + col0 : j * TPB + col0 + C]
                       for j in range(npair)]
                s1v = [s1w[:, j * TPB + col0 : j * TPB + col0 + C]
                       for j in range(npair)]

                X = wt()
                for j, src in enumerate(srcs):
                    nc.any.tensor_copy(X[:, sl[j]], src[:, cs])
                a0 = wt()
                nc.vector.tensor_scalar(a0[:], X[:], 0xFF, None, AND)
                a1 = wt()
                nc.vector.tensor_scalar(a1[:], X[:], 8, None, LSR)
                t00, t10, t01, t11 = wt(), wt(), wt(), wt()
                for tt_, aa, ssv in ((t00, a0, s0v), (t10, a1, s0v),
                                     (t01, a0, s1v), (t11, a1, s1v)):
                    for j in range(npair):
                        nc.vector.tensor_tensor(tt_[:, sl[j]], aa[:, sl[j]],
                                                ssv[j], MULT)
                Apt = wt()
                nc.vector.tensor_scalar(Apt[:], t10[:], 0xFF, 8, AND, LSL)
                v0a = wt()
                nc.vector.tensor_scalar(v0a[:], t00[:], 0xFFFF, None, AND)
                v0 = wt()
                nc.vector.tensor_tensor(v0[:], v0a[:], Apt[:], ADD)
                L0 = wt()
                nc.vector.tensor_scalar(L0[:], v0[:], 0xFFFF, None, AND)
                c0 = wt()
                nc.vector.tensor_scalar(c0[:], v0[:], 16, None, LSR)
                u1a = wt()
                nc.vector.tensor_scalar(u1a[:], t10[:], 8, None, LSR)
                u1 = wt()
                nc.vector.tensor_tensor(u1[:], u1a[:], c0[:], ADD)
                u2a = wt()
                nc.vector.tensor_scalar(u2a[:], t01[:], 0xFFFF, None, AND)
                u2 = wt()
                nc.vector.tensor_tensor(u2[:], u2a[:], u1[:], ADD)
                u3a = wt()
                nc.vector.tensor_scalar(u3a[:], t00[:], 16, None, LSR)
                v1 = wt()
                nc.vector.tensor_tensor(v1[:], u3a[:], u2[:], ADD)
                Ff = wt()
                nc.vector.tensor_scalar(Ff[:], t11[:], 0xFF, 8, AND, LSL)
                v1b = wt()
                nc.vector.tensor_tensor(v1b[:], v1[:], Ff[:], ADD)
                L1 = wt()
                nc.vector.tensor_scalar(L1[:], v1b[:], 0xFFFF, None, AND)
                c1 = wt()
                nc.vector.tensor_scalar(c1[:], v1b[:], 16, None, LSR)
                v2a = wt()
                nc.vector.tensor_scalar(v2a[:], t01[:], 16, None, LSR)
                v2 = wt()
                nc.vector.tensor_tensor(v2[:], v2a[:], c1[:], ADD)
                L2a = wt()
                nc.vector.tensor_scalar(L2a[:], t11[:], 8, None, LSR)
                L2 = wt()
                nc.vector.tensor_tensor(L2[:], L2a[:], v2[:], ADD)

                # xor across pairs -> H limbs [P, C]
                H = []
                for Lt in (L0, L1, L2):
                    Ht = st()
                    nc.vector.tensor_tensor(Ht[:], Lt[:, sl[0]], Lt[:, sl[1]],
                                            XOR)
                    for j in range(2, npair):
                        nc.vector.tensor_tensor(Ht[:], Ht[:], Lt[:, sl[j]],
                                                XOR)
                    H.append(Ht)
                H0, H1, H2 = H

                def mod_m(x):
                    y = st(f32)
                    nc.vector.tensor_tensor(y[:], x[:], INV, MULT)
                    y2 = st(f32)
                    nc.vector.tensor_scalar(y2[:], y[:], 0.5, None, SUB)
                    q = st()
                    nc.vector.tensor_copy(q[:], y2[:])
                    qm = st()
                    nc.vector.tensor_tensor(qm[:], q[:], Mt, MULT)
                    r = st()
                    nc.vector.tensor_tensor(r[:], x[:], qm[:], SUB)
                    ge = st()
                    nc.vector.tensor_tensor(ge[:], r[:], Mt, GE)
                    gm = st()
                    nc.vector.tensor_tensor(gm[:], ge[:], Mt, MULT)
                    r2 = st()
                    nc.vector.tensor_tensor(r2[:], r[:], gm[:], SUB)
                    return r2

                H1a = st()
                nc.vector.tensor_scalar(H1a[:], H1[:], 0xFF, None, AND)
                H1b = st()
                nc.vector.tensor_scalar(H1b[:], H1[:], 8, None, LSR)
                H2a = st()
                nc.vector.tensor_scalar(H2a[:], H2[:], 0xFF, None, AND)
                H2b = st()
                nc.vector.tensor_scalar(H2b[:], H2[:], 8, None, LSR)
                ps = []
                for piece, R in ((H1a, R16), (H1b, R24), (H2a, R32),
                                 (H2b, R40)):
                    pp = st()
                    nc.vector.tensor_tensor(pp[:], piece[:], R, MULT)
                    ps.append(mod_m(pp))
                x1 = st()
                nc.vector.tensor_tensor(x1[:], H0[:], ps[0][:], ADD)
                x2 = st()
                nc.vector.tensor_tensor(x2[:], ps[1][:], ps[2][:], ADD)
                x3 = st()
                nc.vector.tensor_tensor(x3[:], x1[:], x2[:], ADD)
                x4 = st()
                nc.vector.tensor_tensor(x4[:], x3[:], ps[3][:], ADD)
                return mod_m(x4)

            # ---- n2: hash all tokens (grouped-wrapped order), idx -> int16
            idx2_16 = cp.tile([P, TPB], dt.int16, tag="idx2_16", name="idx2_16")
            idx2 = hash_idx([prv_2, cur_2], s0w2, s1w2, cst2, inv2,
                            TPB, 0, "n2")
            nc.vector.tensor_copy(idx2_16[:], idx2[:])

            # ---- n3: hash in slabs so the first gathers start early
            idx3 = cp.tile([P, TPB], i32, tag="idx3", name="idx3")
            C3 = TPB // N3C
            for ch in range(N3C):
                c0_ = ch * C3
                r = hash_idx([pv2_3, prv_3, cur_3], s0w3, s1w3, cst3, inv3,
                             C3, c0_, "n3")
                nc.vector.tensor_copy(idx3[:, c0_ : c0_ + C3], r[:])

            # ---- n2 gathers: 8 dma_gather chunks of 4096 rows
            out2v = out2d.ap().rearrange("(p t) d -> p t d", p=P)
            for a in range(NCH):
                stg = gp.tile([P, TPB], dt.int16, tag="stg", bufs=2,
                              name=f"stg{a}")
                nc.sync.dma_start(stg[0:16, :],
                                  idx2_16[16 * a : 16 * a + 16, :])
                nc.sync.dma_start(stg[16:32, :],
                                  idx2_16[16 * a : 16 * a + 16, :])
                d2 = gp.tile([P, CW * E2], f32, tag="d2", bufs=2,
                             name=f"d2_{a}")
                nc.gpsimd.dma_gather(
                    d2[:].rearrange("p (b e) -> p b e", e=E2),
                    tbl2.ap(),
                    stg[:],
                    4096,
                    4096,
                    E2,
                    single_packet=False,
                )
                nc.sync.dma_start(
                    out2v[:, CW * a : CW * (a + 1), :],
                    d2[:].rearrange("p (b e) -> p b e", e=E2),
                )

            # ---- n3 gathers: one 128-row indirect DMA per token column
            out3v = out3d.ap().rearrange("(p t) d -> p t d", p=P)
            for it in range(TPB // TT3):
                d3 = gp.tile([P, TT3 * SLOT], f32, tag="d3", bufs=3,
                             name=f"d3_{it}")
                for tt in range(TT3):
                    col = it * TT3 + tt
                    nc.gpsimd.indirect_dma_start(
                        out=d3[:, tt * SLOT : (tt + 1) * SLOT],
                        out_offset=None,
                        in_=tbl3.ap(),
                        in_offset=bass.IndirectOffsetOnAxis(
                            ap=idx3[:, col : col + 1], axis=0
                        ),
                    )
                nc.sync.dma_start(
                    out3v[:, it * TT3 : (it + 1) * TT3, :],
                    d3[:].rearrange("p (t d) -> p t d", d=SLOT),
                )

    nc.compile()
    return nc


def _get_nc():
    global _NC
    if _NC is None:
        _NC = _build_nc()
    return _NC


def _broadcast_rows(row):
    return np.ascontiguousarray(np.broadcast_to(row, (P, row.shape[0])))


def _mod_consts(m):
    return [m, 2**16 % m, 2**24 % m, 2**32 % m, 2**40 % m]


def _make_in_maps(inputs):
    ids = np.asarray(inputs["canonical_ids"]).astype(np.int32)  # [B, S]
    hs = np.asarray(inputs["hash_seeds"]).astype(np.int64)      # [3, 8]
    cur = ids.reshape(-1)
    prv = np.pad(ids, ((0, 0), (1, 0)))[:, :S].reshape(-1)
    pv2 = np.pad(ids, ((0, 0), (2, 0)))[:, :S].reshape(-1)
    ids3 = np.ascontiguousarray(np.stack([pv2, prv, cur]).astype(np.int32))
    ids2 = np.ascontiguousarray(
        np.stack([prv[TAU2], cur[TAU2]]).reshape(2, NTOK).astype(np.int32)
    )

    maps = []
    for c in range(8):
        s0, s1, s2 = int(hs[0, c]), int(hs[1, c]), int(hs[2, c])
        m2, m3 = N2_SIZES[c], N3_SIZES[c]

        def seed_row(seeds, lo):
            return np.concatenate(
                [np.full(TPB, (sd & 0xFFFF) if lo else (sd >> 16), np.int32)
                 for sd in seeds]
            )

        cst3row = np.concatenate(
            [np.full(TPB, v, np.int32) for v in _mod_consts(m3)]
        )
        cst2row = np.concatenate(
            [np.full(TPB, v, np.int32) for v in _mod_consts(m2)]
        )
        inv3row = np.full(TPB, np.float64(1.0 / m3) * (1 - 1e-6), np.float32)
        inv2row = np.full(TPB, np.float64(1.0 / m2) * (1 - 1e-6), np.float32)

        tbl3 = np.zeros((V3, SLOT), np.float32)
        w3 = np.asarray(inputs[f"w_n3_h{c}"], dtype=np.float32)
        tbl3[: w3.shape[0]] = w3
        tbl2 = np.zeros((V2, E2), np.float32)
        w2 = np.asarray(inputs[f"w_n2_h{c}"], dtype=np.float32)
        tbl2[: w2.shape[0], :SLOT] = w2

        maps.append(
            {
                "tbl3": tbl3,
                "tbl2": tbl2,
                "ids3": ids3,
                "ids2": ids2,
                "s0w3": _broadcast_rows(seed_row([s0, s1, s2], True)),
                "s1w3": _broadcast_rows(seed_row([s0, s1, s2], False)),
                "s0w2": _broadcast_rows(seed_row([s0, s1], True)),
                "s1w2": _broadcast_rows(seed_row([s0, s1], False)),
                "cst3": _broadcast_rows(cst3row),
                "cst2": _broadcast_rows(cst2row),
                "inv3": _broadcast_rows(inv3row),
                "inv2": _broadcast_rows(inv2row),
            }
        )
    return maps


def kernel(**inputs):
    global LAST_RESULT
    nc = _get_nc()
    in_maps = _make_in_maps(inputs)
    res = run_bass_kernel_spmd(nc, in_maps, core_ids=list(range(8)),
                               trace=TRACE)
    LAST_RESULT = res
    out = np.empty((B, S, 16 * SLOT), np.float32)
    for c in range(8):
        o2 = res.results[c]["out2"].reshape(B, S, E2)
        o3 = res.results[c]["out3"].reshape(B, S, SLOT)
        out[:, :, c * SLOT : (c + 1) * SLOT] = o2[:, :, :SLOT]
        out[:, :, (8 + c) * SLOT : (9 + c) * SLOT] = o3
    return out
